# revision 1
# baseline (speedup 1.0000x reference)
"""Distributed sparse-MoE (top-1 routing, shared expert FFN) for 8 trn2 NeuronCores.

Math: reference computes
    logits = hidden @ Wg + bg ; probs = softmax(logits)
    best   = argmax(probs)    ; order = stable argsort(best)
    out[t] = (hidden[order[t]] @ We + be) * probs[t, best[t]]

Since every expert shares the same FFN weight `We`, the dispatch permutation
commutes with the matmul:  (hidden[order]) @ We = (hidden @ We)[order].
So each core runs the dense FFN matmul on a contiguous 2048-token shard in
ORIGINAL token order (no all-to-all needed); the router gate is a second tiny
matmul over the same token slabs (stationary Wg, 8 output partitions).  The
host applies the data-dependent permutation + top-1 probability scale while
gathering the 8 shards back into the full output.

Device work per core: [2048, 2048] @ [2048, 2048] FFN + [2048, 2048] @
[2048, 8] gate, both in float32r (tf32 inputs, fp32 accumulate, full PE
rate).  tf32 gate safety was verified against the reference's fp32 argmax on
the actual (seeded) inputs: 0/16384 flips, min tf32 top-2 logit gap 5.8e-5
vs ~3e-6 accumulation-order noise.
"""

import os

import numpy as np

import concourse.bacc as bacc
import concourse.bass as bass
import concourse.mybir as mybir
import concourse.tile as tile
from concourse.bass_utils import run_bass_kernel_spmd

# Problem shape (hardcoded per contract).
B, S, H, E = 4, 4096, 2048, 8
T = B * S            # 16384 tokens
NCORES = 8
TPC = T // NCORES    # 2048 tokens per core
P = 128              # partitions
KT = H // P          # 16 contraction blocks
NW = 512             # matmul moving free-dim (one PSUM bank of fp32)
NMAIN = H // NW      # 4 main n-groups
SLAB = 256           # tokens per x DMA slab (2 m-subtiles)

# Main-matmul dtype: "f32r" (tf32, full PE rate), "f32" (4x slower, exact),
# "bf16" (full rate, ~4e-3 rel err).
DT_MAIN = os.environ.get("MOE_DT", "f32r")
# "device": gate matmul on-device (f32r).  "host": numpy fp32 gate.
GATE = os.environ.get("MOE_GATE", "device")


def _round_tf32(a: np.ndarray) -> np.ndarray:
    """Round fp32 to tf32 (10-bit mantissa), round-to-nearest-even."""
    u = np.ascontiguousarray(a, dtype=np.float32).view(np.uint32)
    r = (u + np.uint32(0xFFF) + ((u >> np.uint32(13)) & np.uint32(1))) & np.uint32(
        0xFFFFE000
    )
    return r.view(np.float32)


def _build(dt_main: str, gate_device: bool) -> bass.Bass:
    # Bacc (not raw Bass): its compile() runs generate_event_semaphores,
    # which splits multi-waits to satisfy TRN2's 1-wait-per-instruction
    # hardware constraint.
    nc = bacc.Bacc(None, target_bir_lowering=False)
    f32 = mybir.dt.float32
    f32r = mybir.dt.float32r
    bf16 = mybir.dt.bfloat16
    mm_dt = {"f32r": f32r, "f32": f32, "bf16": bf16}[dt_main]

    # xr: tokens pre-rounded on host to the matmul dtype's precision.
    xr = nc.dram_tensor("xr", [H, TPC], mm_dt, kind="ExternalInput")
    wm = nc.dram_tensor("wm", [H, H], mm_dt, kind="ExternalInput")
    bc = nc.dram_tensor("bc", [1, H], f32, kind="ExternalInput")
    if gate_device:
        wg = nc.dram_tensor("wg", [H, E], mm_dt, kind="ExternalInput")
        bg = nc.dram_tensor("bg", [E, 1], f32, kind="ExternalInput")
        yg = nc.dram_tensor("yg", [E, TPC], f32, kind="ExternalOutput")
    else:
        sc = nc.dram_tensor("sc", [TPC, 1], f32, kind="ExternalInput")
    yo = nc.dram_tensor("yo", [TPC, H], f32, kind="ExternalOutput")

    xr_r = xr[:].rearrange("(ko ki) t -> ki ko t", ki=P)   # [128, KT, TPC]
    wm_r = wm[:].rearrange("(ko ki) n -> ki ko n", ki=P)   # [128, KT, H]

    with tile.TileContext(nc) as tc:
        with (
            tc.tile_pool(name="wpool", bufs=1) as wpool,
            tc.tile_pool(name="cpool", bufs=1) as cpool,
            tc.tile_pool(name="rpool", bufs=3) as rpool,
            tc.tile_pool(name="opool", bufs=2) as opool,
            tc.tile_pool(name="ogpool", bufs=2) as ogpool,
            tc.tile_pool(name="spool", bufs=4) as spool,
            tc.tile_pool(name="pspool", bufs=7, space="PSUM") as pspool,
            tc.tile_pool(name="psgpool", bufs=1, space="PSUM") as psgpool,
        ):
            # Bias row tile (replicated to all 128 partitions by a 0-stride
            # DMA emitted in the preload sequence below).
            b_sb = cpool.tile([P, H], f32)

            n_slabs = TPC // SLAB
            subs = SLAB // P
            PHA = min(2, n_slabs)  # slabs resident during the warmup phase

            def _fetch_slab(m):
                t = rpool.tile([P, KT, SLAB], mm_dt, tag="xm")
                nc.sync.dma_start(out=t, in_=xr_r[:, :, m * SLAB : (m + 1) * SLAB])
                return t

            def _fetch_scale(m):
                t = spool.tile([P, subs], f32, tag="s")
                nc.sync.dma_start(
                    out=t,
                    in_=sc[m * SLAB : (m + 1) * SLAB, :].rearrange(
                        "(s p) o -> p (s o)", p=P
                    ),
                )
                return t

            def do_group(xm, s_m, m, sub, n):
                ps = pspool.tile([P, NW], f32, tag="ps")
                for k in range(KT):
                    nc.tensor.matmul(
                        ps,
                        xm[:, k, sub * P : (sub + 1) * P],
                        w_sb[:, k, n * NW : (n + 1) * NW],
                        start=(k == 0),
                        stop=(k == KT - 1),
                    )
                o_sb = opool.tile([P, NW], f32, tag="o")
                nc.vector.tensor_add(
                    out=o_sb, in0=ps, in1=b_sb[:, n * NW : (n + 1) * NW]
                )
                if s_m is not None:
                    nc.vector.tensor_scalar_mul(
                        out=o_sb, in0=o_sb, scalar1=s_m[:, sub : sub + 1]
                    )
                t0 = (m * subs + sub) * P
                nc.sync.dma_start(
                    out=yo[t0 : t0 + P, n * NW : (n + 1) * NW], in_=o_sb
                )

            def do_gate(xm, m):
                # Gate: stationary Wg (8 cols), moving = the whole token
                # slab.  Output is logits^T [E, SLAB].
                psg = psgpool.tile([E, SLAB], f32, tag="psg")
                for k in range(KT):
                    nc.tensor.matmul(
                        psg,
                        wg_sb[:, k, :],
                        xm[:, k, :],
                        start=(k == 0),
                        stop=(k == KT - 1),
                    )
                og = ogpool.tile([E, SLAB], f32, tag="og")
                nc.vector.tensor_scalar(
                    out=og,
                    in0=psg,
                    scalar1=bg_sb,
                    scalar2=None,
                    op0=mybir.AluOpType.add,
                )
                nc.sync.dma_start(out=yg[:, m * SLAB : (m + 1) * SLAB], in_=og)

            # DMA order: W chunk 0 and slab 0 split into k-halves (PE's first
            # 8-deep half-group can start after ~half the bytes), bias, slab
            # 1, W chunks 1..3, gate weights, prefetched slab 2.  The n-outer
            # warmup below gives PE chunk-0-only work while chunks 1..3 land,
            # so no dispatch ever blocks on W.
            # PE warmup/bridge bursts: dependency-free bf16 matmuls on a
            # memset tile keep the tensor engine busy (and the HAM pstate
            # warm) across DMA-wait windows where no real matmul is ready.
            dum = cpool.tile([P, 128], mybir.dt.bfloat16)
            nc.vector.memset(dum, 1.0)
            dps = psgpool.tile([P, 128], f32, tag="psg", name="dps")

            def warm(count):
                for _ in range(count):
                    nc.tensor.matmul(dps, dum, dum, start=True, stop=True)

            warm(36)

            # Gate weights first (tiny): phase-A gates then run during the
            # W-stream windows where no main matmul is ready.
            if gate_device:
                wg_sb = wpool.tile([P, KT, E], mm_dt)
                nc.sync.dma_start(
                    out=wg_sb, in_=wg[:].rearrange("(ko ki) e -> ki ko e", ki=P)
                )
                bg_sb = cpool.tile([E, 1], f32)
                nc.sync.dma_start(out=bg_sb, in_=bg[:])

            KH = KT // 2
            w_sb = wpool.tile([P, KT, H], mm_dt)
            xm0 = rpool.tile([P, KT, SLAB], mm_dt, tag="xm", name="xm0")
            # First W chunk + first slab interleaved in fine k-pieces (finest
            # first): PE's first accumulation group starts after ~an eighth
            # of the bytes.
            for klo, khi in ((0, 2), (2, 4), (4, 8), (8, 12), (12, 16)):
                ksl = slice(klo, khi)
                nc.sync.dma_start(out=w_sb[:, ksl, :NW], in_=wm_r[:, ksl, :NW])
                nc.sync.dma_start(out=xm0[:, ksl, :], in_=xr_r[:, ksl, :SLAB])
            xms = {0: xm0}
            for m in range(1, PHA):
                xms[m] = _fetch_slab(m)
            bias_bcast = bass.AP(tensor=bc, offset=0, ap=[[0, P], [1, H]])
            nc.sync.dma_start(out=b_sb, in_=bias_bcast)
            scs = {}
            if not gate_device:
                for m in range(PHA):
                    scs[m] = _fetch_scale(m)
            # Remaining W chunks in k-halves so each n-group can begin on
            # half-K as soon as the first half lands.
            for n in range(1, NMAIN):
                nsl = slice(n * NW, (n + 1) * NW)
                nc.sync.dma_start(out=w_sb[:, :KH, nsl], in_=wm_r[:, :KH, nsl])
                nc.sync.dma_start(out=w_sb[:, KH:, nsl], in_=wm_r[:, KH:, nsl])
            # Early prefetch of the first steady-state slab (own pool slot).
            if n_slabs > PHA:
                xm_next = _fetch_slab(PHA)
                sc_next = _fetch_scale(PHA) if not gate_device else None

            # Phase A: gates first (they only need the slab + wg, filling the
            # early W-stream idle), then the main groups n-outer over the
            # resident warmup slabs.
            if gate_device:
                for m in range(PHA):
                    do_gate(xms[m], m)
            for n in range(NMAIN):
                for m in range(PHA):
                    for sub in range(subs):
                        do_group(xms[m], scs.get(m), m, sub, n)

            # Phase B: steady-state, slab-major, software-pipelined prefetch.
            for m in range(PHA, n_slabs):
                xm, s_m = xm_next, sc_next
                if m + 1 < n_slabs:
                    xm_next = _fetch_slab(m + 1)
                    sc_next = _fetch_scale(m + 1) if not gate_device else None
                for sub in range(subs):
                    for n in range(NMAIN):
                        do_group(xm, s_m, m, sub, n)
                if gate_device:
                    do_gate(xm, m)
    nc.compile()
    return nc


_NC_CACHE: dict = {}


def _get_nc(dt_main: str, gate_device: bool) -> bass.Bass:
    key = (dt_main, gate_device)
    if key not in _NC_CACHE:
        _NC_CACHE[key] = _build(dt_main, gate_device)
    return _NC_CACHE[key]


def _softmax_top1(logits: np.ndarray):
    """best index, top-1 softmax prob (fp32, matches jax argmax semantics)."""
    logits = np.ascontiguousarray(logits, dtype=np.float32)
    mx = logits.max(axis=1, keepdims=True)
    ex = np.exp(logits - mx, dtype=np.float32)
    denom = ex.sum(axis=1)
    best = logits.argmax(axis=1)
    best_p = ex[np.arange(logits.shape[0]), best] / denom
    return best, best_p


def _prep_mm(a: np.ndarray, dt_main: str) -> np.ndarray:
    """Prepare an operand for the main matmul's dtype (host-side rounding)."""
    if dt_main == "f32r":
        return _round_tf32(a)
    if dt_main == "bf16":
        import ml_dtypes

        return np.ascontiguousarray(a).astype(ml_dtypes.bfloat16)
    return np.ascontiguousarray(a)


def kernel(x, Wg, bg, We, be):
    x = np.asarray(x, dtype=np.float32)
    Wg = np.asarray(Wg, dtype=np.float32)
    bg = np.asarray(bg, dtype=np.float32)
    We = np.asarray(We, dtype=np.float32)
    be = np.asarray(be, dtype=np.float32)

    hidden = np.ascontiguousarray(x.reshape(T, H))
    gate_device = GATE == "device"
    nc = _get_nc(DT_MAIN, gate_device)
    wm_np = _prep_mm(We, DT_MAIN)
    bc_np = be[None, :].astype(np.float32)

    if gate_device:
        wg_np = _prep_mm(Wg, DT_MAIN)
        bg_np = np.ascontiguousarray(bg[:, None]).astype(np.float32)
        in_maps = []
        for c in range(NCORES):
            xt_c = np.ascontiguousarray(hidden[c * TPC : (c + 1) * TPC].T)
            in_maps.append(
                {
                    "xr": _prep_mm(xt_c, DT_MAIN),
                    "wm": wm_np,
                    "wg": wg_np,
                    "bc": bc_np,
                    "bg": bg_np,
                }
            )
        res = run_bass_kernel_spmd(nc, in_maps, core_ids=list(range(NCORES)))
        y = np.concatenate([r["yo"] for r in res.results], axis=0)      # [T, H]
        logits = np.concatenate([r["yg"] for r in res.results], axis=1).T
        # Tie guard: the device gate runs at tf32 precision (logit error
        # ~1e-4).  For the few tokens whose top-2 gap is within that bound,
        # recompute the logits exactly (fp64) so a near-tie can never flip
        # the argmax vs the fp32 reference and corrupt the sort permutation.
        logits = np.ascontiguousarray(logits, dtype=np.float32)
        srt = np.sort(logits, axis=1)
        suspects = np.nonzero(srt[:, -1] - srt[:, -2] < 1e-3)[0]
        if suspects.size:
            exact = (
                hidden[suspects].astype(np.float64) @ Wg.astype(np.float64)
                + bg.astype(np.float64)
            ).astype(np.float32)
            logits[suspects] = exact
        best, best_p = _softmax_top1(logits)
        order = np.argsort(best, kind="stable")
        out = y[order] * best_p[:, None]
    else:
        # Host gate: shards are the tokens PERMUTED by destination slot; the
        # device applies the top-1 scale, so shard outputs are final rows.
        logits = hidden @ Wg + bg
        best, best_p = _softmax_top1(logits)
        order = np.argsort(best, kind="stable")
        xp = hidden[order]
        in_maps = []
        for c in range(NCORES):
            xt_c = np.ascontiguousarray(xp[c * TPC : (c + 1) * TPC].T)
            sc_c = np.ascontiguousarray(best_p[c * TPC : (c + 1) * TPC, None])
            in_maps.append(
                {"xr": _prep_mm(xt_c, DT_MAIN), "wm": wm_np, "bc": bc_np, "sc": sc_c}
            )
        res = run_bass_kernel_spmd(nc, in_maps, core_ids=list(range(NCORES)))
        out = np.concatenate([r["yo"] for r in res.results], axis=0)

    return out.reshape(B, S, H).astype(np.float32)



# revision 4
# speedup vs baseline: 1.7075x; 1.7075x over previous
"""Distributed sparse-MoE (top-1 routing, shared expert FFN) for 8 trn2 NeuronCores.

Math: reference computes
    logits = hidden @ Wg + bg ; probs = softmax(logits)
    best   = argmax(probs)    ; order = stable argsort(best)
    out[t] = (hidden[order[t]] @ We + be) * probs[t, best[t]]

Since every expert shares the same FFN weight `We`, the dispatch permutation
commutes with the matmul; the host routes (gate matmul + argsort + permute)
and each core runs the dense FFN matmul on its contiguous 2048-token shard of
the PERMUTED token stream.  The host folds the 2^-6 weight prescale, the FFN
bias and the top-1 probability into one final fused broadcast multiply.

Device math per core — error-compensated fp8 with DoubleRow perf mode:
    x  = X1 + X2   (X1 = e4m3(x),     X2 = e4m3(x - X1))
    We*64 = W1 + W2 (W1 = e4m3(64 We), W2 = e4m3(64 We - W1))
    psum = X1'W1 + X2'W1 + X1'W2[first HB k-blocks]      (one accumulation
    group; all three terms share the 2^6 scale, so no merge passes and the
    output DMAs straight out of PSUM).
X2 everywhere kills the x-quantization error (first order); W2 on HB=8 of 16
k-blocks leaves sqrt(8/16)*2.65% ~= 1.87% L2 error (measured 1.8735e-2 on the
seeded inputs, gate 2e-2).  DoubleRow fp8 matmuls process 2 k-blocks per
instruction at 0.5 cycles/row: 20 instructions per [128,512] output tile vs
32 f32r equivalents -> 0.625x the PE time of the f32r kernel.

Near-tie safety: the host gate computes fp32 logits exactly like the
reference (dense [T,H]@[H,E]); tokens whose top-2 logit gap < 1e-3 are
recomputed in fp64 so accumulation-order noise can never flip the argmax and
corrupt the sort permutation (min top-2 gap on the seeded inputs: 1.9e-5,
noise ~3e-6).
"""

import os

os.environ.setdefault("BASS_NEVER_TRACE", "1")

import ml_dtypes
import numpy as np

import concourse.bacc as bacc
import concourse.bass as bass
import concourse.mybir as mybir
import concourse.tile as tile
from concourse.bass_utils import run_bass_kernel_spmd

# Problem shape (hardcoded per contract).
B, S, H, E = 4, 4096, 2048, 8
T = B * S            # 16384 tokens
NCORES = 8
TPC = T // NCORES    # 2048 tokens per core
P = 128              # partitions
KT = H // P          # 16 contraction blocks
KP = KT // 2         # 8 DoubleRow k-pairs
HB = 8               # k-blocks with W-residual correction (error budget knob)
HBP = HB // 2        # 4 w-corr DoubleRow pairs
NW = 512             # matmul moving free-dim (one PSUM bank of fp32)
NMAIN = H // NW      # 4 n-groups
SLAB = 512           # tokens per x DMA slab
NSLAB = TPC // SLAB  # 4 slabs
SUBS = SLAB // P     # 4 m-subtiles per slab
WSCALE = 64.0        # We prescale (2^6) keeping W1 out of e4m3 subnormals
WARM = 40            # PE warmup dummy matmuls (pstate ramp + DMA cover)

F8NP = ml_dtypes.float8_e4m3

# Kept for test.py compatibility / debugging printouts.
DT_MAIN = "fp8dr"
GATE = "host"


def _build() -> bass.Bass:
    nc = bacc.Bacc(None, target_bir_lowering=False)
    f8 = mybir.dt.float8e4
    f32 = mybir.dt.float32
    DR = mybir.MatmulPerfMode.DoubleRow

    x1 = nc.dram_tensor("x1", [H, TPC], f8, kind="ExternalInput")
    x2 = nc.dram_tensor("x2", [H, TPC], f8, kind="ExternalInput")
    w1 = nc.dram_tensor("w1", [H, H], f8, kind="ExternalInput")
    w2 = nc.dram_tensor("w2", [HB * P, H], f8, kind="ExternalInput")
    yo = nc.dram_tensor("yo", [TPC, H], f32, kind="ExternalOutput")

    x1r = x1[:].rearrange("(ko ki) t -> ki ko t", ki=P)   # [128, KT, TPC]
    x2r = x2[:].rearrange("(ko ki) t -> ki ko t", ki=P)
    w1r = w1[:].rearrange("(ko ki) n -> ki ko n", ki=P)   # [128, KT, H]
    w2r = w2[:].rearrange("(ko ki) n -> ki ko n", ki=P)   # [128, HB, H]

    with tile.TileContext(nc) as tc:
        with (
            tc.tile_pool(name="wpool", bufs=1) as wpool,
            tc.tile_pool(name="cpool", bufs=1) as cpool,
            tc.tile_pool(name="r1pool", bufs=3) as r1pool,
            tc.tile_pool(name="r2pool", bufs=3) as r2pool,
            tc.tile_pool(name="opool", bufs=6) as opool,
            tc.tile_pool(name="pspool", bufs=7, space="PSUM") as pspool,
            tc.tile_pool(name="pwpool", bufs=1, space="PSUM") as pwpool,
        ):
            def _fetch_x1(m):
                t = r1pool.tile([P, KT, SLAB], f8, tag="x1")
                nc.sync.dma_start(out=t, in_=x1r[:, :, m * SLAB : (m + 1) * SLAB])
                return t

            def _fetch_x2(m):
                t = r2pool.tile([P, KT, SLAB], f8, tag="x2")
                nc.sync.dma_start(out=t, in_=x2r[:, :, m * SLAB : (m + 1) * SLAB])
                return t

            def do_group(x1m, x2m, m, sub, n):
                """One [128,512] output tile: 20 DoubleRow matmuls, one PSUM
                group, DMA straight from PSUM (host applies 1/64, bias, p)."""
                ps = pspool.tile([P, NW], f32, tag="ps")
                ssl = slice(sub * P, (sub + 1) * P)
                nsl = slice(n * NW, (n + 1) * NW)
                for kp in range(KP):
                    ksl = slice(2 * kp, 2 * kp + 2)
                    nc.tensor.matmul(
                        ps, x1m[:, ksl, ssl], w1_sb[:, ksl, nsl],
                        start=(kp == 0), stop=False, perf_mode=DR,
                    )
                for kp in range(KP):
                    ksl = slice(2 * kp, 2 * kp + 2)
                    nc.tensor.matmul(
                        ps, x2m[:, ksl, ssl], w1_sb[:, ksl, nsl],
                        start=False, stop=False, perf_mode=DR,
                    )
                for kp in range(HBP):
                    ksl = slice(2 * kp, 2 * kp + 2)
                    nc.tensor.matmul(
                        ps, x1m[:, ksl, ssl], w2_sb[:, ksl, nsl],
                        start=False, stop=(kp == HBP - 1), perf_mode=DR,
                    )
                o_sb = opool.tile([P, NW], f32, tag="o")
                # PSUM can't feed DMA directly; alternate the copy between
                # the idle Activation and Vector engines.
                if (m * SUBS + sub + n) % 2 == 0:
                    nc.scalar.copy(out=o_sb, in_=ps)
                else:
                    nc.vector.tensor_copy(out=o_sb, in_=ps)
                t0 = (m * SUBS + sub) * P
                nc.sync.dma_start(out=yo[t0 : t0 + P, nsl], in_=o_sb)

            # PE warmup: dependency-free bf16 matmuls on a memset tile ramp the
            # pstate and cover the initial W/x DMA latency.
            dum = cpool.tile([P, 128], mybir.dt.bfloat16)
            nc.vector.memset(dum, 1.0)
            dps = pwpool.tile([P, 128], f32, tag="dps")
            for _ in range(WARM):
                nc.tensor.matmul(dps, dum, dum, start=True, stop=True)

            w1_sb = wpool.tile([P, KT, H], f8)
            w2_sb = wpool.tile([P, HB, H], f8)
            x1s = {0: r1pool.tile([P, KT, SLAB], f8, tag="x1", name="x1s0")}
            x2s = {0: r2pool.tile([P, KT, SLAB], f8, tag="x2", name="x2s0")}

            # Startup DMA order: first W1 n0-chunk and x1 slab 0 in fine
            # k-pieces (PE's first accumulation group starts after ~a quarter
            # of the bytes), then x2 slab 0 + w2 n0, then the remaining W
            # n-chunks, then slab prefetches.
            nsl0 = slice(0, NW)
            for klo, khi in ((0, 2), (2, 4), (4, 8), (8, 12), (12, 16)):
                ksl = slice(klo, khi)
                nc.sync.dma_start(out=w1_sb[:, ksl, nsl0], in_=w1r[:, ksl, nsl0])
                nc.sync.dma_start(out=x1s[0][:, ksl, :], in_=x1r[:, ksl, :SLAB])
            nc.sync.dma_start(out=x2s[0], in_=x2r[:, :, :SLAB])
            nc.sync.dma_start(out=w2_sb[:, :, nsl0], in_=w2r[:, :, nsl0])
            for n in range(1, NMAIN):
                nsl = slice(n * NW, (n + 1) * NW)
                nc.sync.dma_start(out=w1_sb[:, :KP, nsl], in_=w1r[:, :KP, nsl])
                nc.sync.dma_start(out=w1_sb[:, KP:, nsl], in_=w1r[:, KP:, nsl])
                nc.sync.dma_start(out=w2_sb[:, :, nsl], in_=w2r[:, :, nsl])
            x1s[1] = _fetch_x1(1)
            x2s[1] = _fetch_x2(1)
            x1s[2] = _fetch_x1(2)
            x2s[2] = _fetch_x2(2)

            # Slab 0 runs n-outer so the PE only needs the n0 W-chunk to get
            # going while n1..n3 stream in.
            for n in range(NMAIN):
                for sub in range(SUBS):
                    do_group(x1s[0], x2s[0], 0, sub, n)

            # Steady state: slab-major, prefetch one slab ahead.
            for m in range(1, NSLAB):
                if m + 2 < NSLAB:
                    x1s[m + 2] = _fetch_x1(m + 2)
                    x2s[m + 2] = _fetch_x2(m + 2)
                for sub in range(SUBS):
                    for n in range(NMAIN):
                        do_group(x1s[m], x2s[m], m, sub, n)
    nc.compile()
    return nc


_NC_CACHE: dict = {}


def _get_nc(*_args) -> bass.Bass:
    if "nc" not in _NC_CACHE:
        _NC_CACHE["nc"] = _build()
    return _NC_CACHE["nc"]


def _q8(a: np.ndarray) -> np.ndarray:
    """Round fp32 -> e4m3 (RTNE), keeping the ml_dtypes array for upload."""
    return np.ascontiguousarray(a).astype(F8NP)


def _route(hidden: np.ndarray, Wg: np.ndarray, bg: np.ndarray):
    """fp32 gate matmul + argmax with fp64 tie-guard + stable sort, exactly
    mirroring the reference's routing semantics."""
    logits = hidden @ Wg + bg
    srt = np.sort(logits, axis=1)
    suspects = np.nonzero(srt[:, -1] - srt[:, -2] < 1e-3)[0]
    if suspects.size:
        exact = (
            hidden[suspects].astype(np.float64) @ Wg.astype(np.float64)
            + bg.astype(np.float64)
        ).astype(np.float32)
        logits[suspects] = exact
    mx = logits.max(axis=1, keepdims=True)
    ex = np.exp(logits - mx, dtype=np.float32)
    best = logits.argmax(axis=1)
    best_p = ex[np.arange(logits.shape[0]), best] / ex.sum(axis=1)
    order = np.argsort(best, kind="stable")
    return order, best_p


def kernel(x, Wg, bg, We, be):
    x = np.asarray(x, dtype=np.float32)
    Wg = np.asarray(Wg, dtype=np.float32)
    bg = np.asarray(bg, dtype=np.float32)
    We = np.asarray(We, dtype=np.float32)
    be = np.asarray(be, dtype=np.float32)

    hidden = np.ascontiguousarray(x.reshape(T, H))
    order, best_p = _route(hidden, Wg, bg)
    xp = hidden[order]                                   # dispatched tokens

    ws = We * WSCALE
    w1_np = _q8(ws)
    w2_np = _q8((ws - w1_np.astype(np.float32))[: HB * P])

    nc = _get_nc()
    in_maps = []
    for c in range(NCORES):
        xt = np.ascontiguousarray(xp[c * TPC : (c + 1) * TPC].T)  # [H, TPC]
        x1_np = _q8(xt)
        x2_np = _q8(xt - x1_np.astype(np.float32))
        in_maps.append({"x1": x1_np, "x2": x2_np, "w1": w1_np, "w2": w2_np})
    res = run_bass_kernel_spmd(nc, in_maps, core_ids=list(range(NCORES)))
    y = np.concatenate([r["yo"] for r in res.results], axis=0)    # [T, H]

    out = (y * np.float32(1.0 / WSCALE) + be) * best_p[:, None]
    return out.reshape(B, S, H).astype(np.float32)


# revision 27
# speedup vs baseline: 1.8089x; 1.0594x over previous
"""Distributed sparse-MoE (top-1 routing, shared expert FFN) for 8 trn2 NeuronCores.

Math: reference computes
    logits = hidden @ Wg + bg ; probs = softmax(logits)
    best   = argmax(probs)    ; order = stable argsort(best)
    out[t] = (hidden[order[t]] @ We + be) * probs[t, best[t]]

Since every expert shares the same FFN weight `We`, the dispatch permutation
commutes with the matmul; the host routes (gate matmul + argsort + permute)
and each core runs the dense FFN matmul on its contiguous 2048-token shard of
the PERMUTED token stream.  The host folds the 2^-6 weight prescale, the FFN
bias and the top-1 probability into one final fused broadcast multiply.

Device math per core — error-compensated fp8 with DoubleRow perf mode:
    x  = X1 + X2   (X1 = e4m3(x),     X2 = e4m3(x - X1))
    We*64 = W1 + W2 (W1 = e4m3(64 We), W2 = e4m3(64 We - W1))
    psum = X1'W1 + X2'W1 + X1'W2[first HB k-blocks]      (one accumulation
    group; all three terms share the 2^6 scale, so no merge passes and the
    output DMAs straight out of PSUM).
X2 everywhere kills the x-quantization error (first order); W2 on HB=8 of 16
k-blocks leaves sqrt(8/16)*2.65% ~= 1.87% L2 error (measured 1.8735e-2 on the
seeded inputs, gate 2e-2).  DoubleRow fp8 matmuls process 2 k-blocks per
instruction at 0.5 cycles/row: 20 instructions per [128,512] output tile vs
32 f32r equivalents -> 0.625x the PE time of the f32r kernel.

Near-tie safety: the host gate computes fp32 logits exactly like the
reference (dense [T,H]@[H,E]); tokens whose top-2 logit gap < 1e-3 are
recomputed in fp64 so accumulation-order noise can never flip the argmax and
corrupt the sort permutation (min top-2 gap on the seeded inputs: 1.9e-5,
noise ~3e-6).
"""

import os

os.environ.setdefault("BASS_NEVER_TRACE", "1")

import ml_dtypes
import numpy as np

import concourse.bacc as bacc
import concourse.bass as bass
import concourse.mybir as mybir
import concourse.tile as tile
from concourse.bass_utils import run_bass_kernel_spmd

# Problem shape (hardcoded per contract).
B, S, H, E = 4, 4096, 2048, 8
T = B * S            # 16384 tokens
NCORES = 8
TPC = T // NCORES    # 2048 tokens per core
P = 128              # partitions
KT = H // P          # 16 contraction blocks
KP = KT // 2         # 8 DoubleRow k-pairs
HB = 8               # k-blocks with W-residual correction (error budget knob)
HBP = HB // 2        # 4 w-corr DoubleRow pairs
NW = 512             # matmul moving free-dim (one PSUM bank of fp32)
NMAIN = H // NW      # 4 n-groups
SLAB = 512           # tokens per x DMA slab
NSLAB = TPC // SLAB  # 4 slabs
SUBS = SLAB // P     # 4 m-subtiles per slab
WSCALE = 64.0        # We prescale (2^6) keeping W1 out of e4m3 subnormals
WARM = 40            # PE warmup dummy matmuls (pstate ramp + DMA cover)

F8NP = ml_dtypes.float8_e4m3

# Kept for test.py compatibility / debugging printouts.
DT_MAIN = "fp8dr"
GATE = "host"


def _build() -> bass.Bass:
    nc = bacc.Bacc(None, target_bir_lowering=False)
    f8 = mybir.dt.float8e4
    f32 = mybir.dt.float32
    DR = mybir.MatmulPerfMode.DoubleRow

    bf16 = mybir.dt.bfloat16
    x1 = nc.dram_tensor("x1", [H, TPC], f8, kind="ExternalInput")
    x2 = nc.dram_tensor("x2", [H, TPC], f8, kind="ExternalInput")
    w1 = nc.dram_tensor("w1", [H, H], f8, kind="ExternalInput")
    w2 = nc.dram_tensor("w2", [HB * P, H], f8, kind="ExternalInput")
    # bf16 output halves the writeback traffic (adds ~0.1% error in
    # quadrature against the 1.87% fp8 budget); host upcasts.
    yo = nc.dram_tensor("yo", [TPC, H], bf16, kind="ExternalOutput")

    x1r = x1[:].rearrange("(ko ki) t -> ki ko t", ki=P)   # [128, KT, TPC]
    x2r = x2[:].rearrange("(ko ki) t -> ki ko t", ki=P)
    w1r = w1[:].rearrange("(ko ki) n -> ki ko n", ki=P)   # [128, KT, H]
    w2r = w2[:].rearrange("(ko ki) n -> ki ko n", ki=P)   # [128, HB, H]

    with tile.TileContext(nc) as tc:
        with (
            tc.tile_pool(name="wpool", bufs=1) as wpool,
            tc.tile_pool(name="cpool", bufs=1) as cpool,
            tc.tile_pool(name="r1pool", bufs=3) as r1pool,
            tc.tile_pool(name="r2pool", bufs=3) as r2pool,
            tc.tile_pool(name="opool", bufs=8) as opool,
            tc.tile_pool(name="pspool", bufs=7, space="PSUM") as pspool,
            tc.tile_pool(name="pwpool", bufs=1, space="PSUM") as pwpool,
        ):
            def _fetch_x1(m):
                t = r1pool.tile([P, KT, SLAB], f8, tag="x1")
                nc.sync.dma_start(out=t, in_=x1r[:, :, m * SLAB : (m + 1) * SLAB])
                return t

            def _fetch_x2(m):
                t = r2pool.tile([P, KT, SLAB], f8, tag="x2")
                nc.sync.dma_start(out=t, in_=x2r[:, :, m * SLAB : (m + 1) * SLAB])
                return t

            def do_group(x1m, x2m, m, sub, n, nsl=None, nw=NW, eng=None, ring=None):
                """One [128,nw] output tile: 20 DoubleRow matmuls, one PSUM
                group, copy to SBUF, DMA out (host applies 1/64, bias, p)."""
                ps = pspool.tile([P, nw], f32, tag="ps")
                ssl = slice(sub * P, (sub + 1) * P)
                if nsl is None:
                    nsl = slice(n * NW, (n + 1) * NW)
                for kp in range(KP):
                    ksl = slice(2 * kp, 2 * kp + 2)
                    nc.tensor.matmul(
                        ps, x1m[:, ksl, ssl], w1_sb[:, ksl, nsl],
                        start=(kp == 0), stop=False, perf_mode=DR,
                    )
                for kp in range(KP):
                    ksl = slice(2 * kp, 2 * kp + 2)
                    nc.tensor.matmul(
                        ps, x2m[:, ksl, ssl], w1_sb[:, ksl, nsl],
                        start=False, stop=False, perf_mode=DR,
                    )
                for kp in range(HBP):
                    ksl = slice(2 * kp, 2 * kp + 2)
                    nc.tensor.matmul(
                        ps, x1m[:, ksl, ssl], w2_sb[:, ksl, nsl],
                        start=False, stop=(kp == HBP - 1), perf_mode=DR,
                    )
                o_sb = opool.tile([P, nw], bf16, tag="o")
                # PSUM can't feed DMA directly; alternate the copy between
                # the idle Activation and Vector engines.  Output DMAs go out
                # on the Activation HWDGE ring so they interleave with (not
                # queue behind) the SP-ring input prefetch stream.
                if eng is None:
                    eng = (m * SUBS + sub + n) % 2
                if eng == 0:
                    nc.scalar.copy(out=o_sb, in_=ps)
                else:
                    nc.vector.tensor_copy(out=o_sb, in_=ps)
                t0 = (m * SUBS + sub) * P
                (ring or nc.scalar).dma_start(out=yo[t0 : t0 + P, nsl], in_=o_sb)

            # PE warmup: dependency-free bf16 matmuls on a memset tile ramp the
            # pstate and cover the initial W/x DMA latency.
            dum = cpool.tile([P, 128], mybir.dt.bfloat16)
            nc.gpsimd.memset(dum, 1.0)
            dps = pwpool.tile([P, 128], f32, tag="dps")
            for _ in range(WARM):
                nc.tensor.matmul(dps, dum, dum, start=True, stop=True)

            w1_sb = wpool.tile([P, KT, H], f8)
            w2_sb = wpool.tile([P, HB, H], f8)
            x1s = {0: r1pool.tile([P, KT, SLAB], f8, tag="x1", name="x1s0")}
            x2s = {0: r2pool.tile([P, KT, SLAB], f8, tag="x2", name="x2s0")}

            # Startup DMAs split across BOTH HWDGE rings so issue rate
            # (~630ns/DMA/ring) doesn't serialize the stream: the SP ring
            # interleaves W1-n0 and x1-slab0 in fine k-pieces (the first
            # accumulation group starts after ~a fifth of the bytes), the
            # otherwise-idle Activation ring carries x2-slab0 and w2-n0 in
            # parallel so the xcorr/wcorr matmuls are ready in time.
            nsl0 = slice(0, NW)
            rings = [nc.sync, nc.scalar]
            ri = 0

            def _start_dma(out, in_):
                nonlocal ri
                rings[ri % 2].dma_start(out=out, in_=in_)
                ri += 1

            # Pieces in global NEED order, alternating rings only to double
            # the HWDGE issue rate (~630ns per DMA per ring).
            for klo, khi in ((0, 2), (2, 4), (4, 8), (8, 12), (12, 16)):
                ksl = slice(klo, khi)
                _start_dma(w1_sb[:, ksl, nsl0], w1r[:, ksl, nsl0])
                _start_dma(x1s[0][:, ksl, :], x1r[:, ksl, :SLAB])
            _start_dma(x2s[0][:, :KP, :], x2r[:, :KP, :SLAB])
            _start_dma(x2s[0][:, KP:, :], x2r[:, KP:, :SLAB])
            _start_dma(w2_sb[:, :, nsl0], w2r[:, :, nsl0])
            for n in range(1, NMAIN):
                nsl = slice(n * NW, (n + 1) * NW)
                nc.sync.dma_start(out=w1_sb[:, :KP, nsl], in_=w1r[:, :KP, nsl])
                nc.sync.dma_start(out=w1_sb[:, KP:, nsl], in_=w1r[:, KP:, nsl])
                nc.sync.dma_start(out=w2_sb[:, :, nsl], in_=w2r[:, :, nsl])
            x1s[1] = _fetch_x1(1)
            x2s[1] = _fetch_x2(1)
            x1s[2] = _fetch_x1(2)
            x2s[2] = _fetch_x2(2)

            # Slab 0 runs n-outer so the PE only needs the n0 W-chunk to get
            # going while n1..n3 stream in.
            for n in range(NMAIN):
                for sub in range(SUBS):
                    do_group(x1s[0], x2s[0], 0, sub, n)

            # Steady state: slab-major, prefetch one slab ahead.  The very
            # last group is emitted as two half-width PSUM groups so its
            # copy+writeback overlaps the final matmuls (shorter tail drain).
            for m in range(1, NSLAB):
                if m + 2 < NSLAB:
                    x1s[m + 2] = _fetch_x1(m + 2)
                    x2s[m + 2] = _fetch_x2(m + 2)
                for sub in range(SUBS):
                    for n in range(NMAIN):
                        last = m == NSLAB - 1 and sub == SUBS - 1 and n == NMAIN - 1
                        if not last:
                            do_group(x1s[m], x2s[m], m, sub, n)
                        else:
                            # Final output tile in 4 staggered quarter-width
                            # groups: the copy+writeback+semaphore chain of
                            # quarters 0..2 overlaps the remaining matmuls,
                            # shortening the end-of-kernel drain.
                            qw = NW // 4
                            for q in range(4):
                                nsl = slice(n * NW + q * qw, n * NW + (q + 1) * qw)
                                do_group(
                                    x1s[m], x2s[m], m, sub, n,
                                    nsl=nsl, nw=qw, eng=q % 2,
                                    ring=(nc.scalar if q % 2 == 0 else nc.sync),
                                )
    nc.compile()
    return nc


_NC_CACHE: dict = {}


def _get_nc(*_args) -> bass.Bass:
    if "nc" not in _NC_CACHE:
        _NC_CACHE["nc"] = _build()
    return _NC_CACHE["nc"]


def _q8(a: np.ndarray) -> np.ndarray:
    """Round fp32 -> e4m3 (RTNE), keeping the ml_dtypes array for upload."""
    return np.ascontiguousarray(a).astype(F8NP)


def _route(hidden: np.ndarray, Wg: np.ndarray, bg: np.ndarray):
    """fp32 gate matmul + argmax with fp64 tie-guard + stable sort, exactly
    mirroring the reference's routing semantics."""
    logits = hidden @ Wg + bg
    srt = np.sort(logits, axis=1)
    suspects = np.nonzero(srt[:, -1] - srt[:, -2] < 1e-3)[0]
    if suspects.size:
        exact = (
            hidden[suspects].astype(np.float64) @ Wg.astype(np.float64)
            + bg.astype(np.float64)
        ).astype(np.float32)
        logits[suspects] = exact
    mx = logits.max(axis=1, keepdims=True)
    ex = np.exp(logits - mx, dtype=np.float32)
    best = logits.argmax(axis=1)
    best_p = ex[np.arange(logits.shape[0]), best] / ex.sum(axis=1)
    order = np.argsort(best, kind="stable")
    return order, best_p


def kernel(x, Wg, bg, We, be):
    x = np.asarray(x, dtype=np.float32)
    Wg = np.asarray(Wg, dtype=np.float32)
    bg = np.asarray(bg, dtype=np.float32)
    We = np.asarray(We, dtype=np.float32)
    be = np.asarray(be, dtype=np.float32)

    hidden = np.ascontiguousarray(x.reshape(T, H))
    order, best_p = _route(hidden, Wg, bg)
    xp = hidden[order]                                   # dispatched tokens

    ws = We * WSCALE
    w1_np = _q8(ws)
    w2_np = _q8((ws - w1_np.astype(np.float32))[: HB * P])

    nc = _get_nc()
    in_maps = []
    for c in range(NCORES):
        xt = np.ascontiguousarray(xp[c * TPC : (c + 1) * TPC].T)  # [H, TPC]
        x1_np = _q8(xt)
        x2_np = _q8(xt - x1_np.astype(np.float32))
        in_maps.append({"x1": x1_np, "x2": x2_np, "w1": w1_np, "w2": w2_np})
    res = run_bass_kernel_spmd(nc, in_maps, core_ids=list(range(NCORES)))
    y = np.concatenate(
        [np.asarray(r["yo"]).astype(np.float32) for r in res.results], axis=0
    )                                                             # [T, H]

    out = (y * np.float32(1.0 / WSCALE) + be) * best_p[:, None]
    return out.reshape(B, S, H).astype(np.float32)
